# revision 1
# baseline (speedup 1.0000x reference)
"""AdaBlock (binarized double-conv residual block) Trainium2 kernel.

Strategy
--------
Data-parallel over batch: 16 images across 8 NeuronCores (2 images/core), no
collectives.  The binarized convs are exact +-1 matmuls: a 3x3 conv is 9
shifted [Cin x spatial] matmuls accumulated in PSUM.  fp8 with
`perf_mode=DoubleRow` packs both 128-channel cin halves into one K=256
matmul (~90 ns / N=462 matmul measured on HW).  Sign activations are
computed as (x >= -b) - 0.5 on the VectorE (values +-0.5, exact in fp8; the
factor 2 is folded into the per-out-channel conv scales), intermediates are
fp16 to hit the DVE 2x/4x perf modes, PSUM accumulation stays fp32 so conv
sums are exact; overall mean rel err vs the fp32 reference ~2e-3.

Spatial layout: sign activations live in a zero-ring-padded 66x66 grid per
cin half (flat, half-stride 4368 so the DoubleRow rhs AP is [p, 2, N]).
Conv output is tiled over 7 padded rows (N=462) per PSUM bank (+ a 1-row
runt); the kx tap shift is applied to the PSUM output AP (window 2-kx) so
every rhs offset stays even, and each drain is one strided op into the flat
64x64 layout.  Row-tiles are grouped 4/4/2 so each stationary weight load
feeds 4 matmuls.

Per-core pipeline (per image):
  DMA x (fp16, converted on host) -> s1 = sign8(x + bias1_)
  conv1: 2 outgrps x 10 row-tiles x 9 taps DoubleRow matmuls -> PSUM
  t1    = psum * sc1 (ScalarE, fp16); xres = x + t1      (in-place, DVE)
  s2[h] = sign8(xres[h] + bias2_)   (emitted per group, overlaps group h+1)
  conv2: 10 row-tiles x 9 taps -> PSUM
  t2    = psum * sc2 + bias3;  u = xres[:128] + t2;  u = prelu(u)
  pixel-unshuffle: 4 strided ScalarE copies (+bias4, cast f32) -> DMA out
"""

import numpy as np
import ml_dtypes

import concourse.bass as bass
import concourse.mybir as mybir
from concourse import bacc
from concourse.tile import TileContext
from concourse.bass_utils import run_bass_kernel_spmd

B, C, H, W = 16, 256, 64, 64
NCORES = 8
BL = B // NCORES          # images per core
HW_ = H * W               # 4096
PW = W + 2                # 66 padded row width
HS = 4368                 # per-half stride in the sign buffer (16-aligned)
GRID = 0                  # padded 66x66 grid starts at this offset in a half
F32 = mybir.dt.float32
FP16 = mybir.dt.float16
FP8 = mybir.dt.float8e4
DR = mybir.MatmulPerfMode.DoubleRow

# row-tiles: 9 tiles of 7 output rows + 1 runt row
TILES = [(t * 7, 7) for t in range(9)] + [(63, 1)]

_CACHE = {}


def build_nc(reps=1, probe=None):
    nc = bacc.Bacc()
    x_ext = nc.declare_dram_parameter("x", [BL, C, H, W], FP16, isOutput=False)
    w1_ext = nc.declare_dram_parameter("w1", [128, 18 * 256], FP8, isOutput=False)
    w2_ext = nc.declare_dram_parameter("w2", [128, 9 * 256], FP8, isOutput=False)
    coef_ext = nc.declare_dram_parameter("coef", [128, 10], F32, isOutput=False)
    out_ext = nc.declare_dram_parameter("out", [BL, 2 * C, H // 2, W // 2], F32,
                                        isOutput=True)

    Ident = mybir.ActivationFunctionType.Identity
    Alu = mybir.AluOpType

    with TileContext(nc) as tc:
        with (
            tc.tile_pool(name="weights", bufs=1) as pw,
            tc.tile_pool(name="xbuf", bufs=8) as px,
            tc.tile_pool(name="signs", bufs=6) as psn,
            tc.tile_pool(name="small", bufs=8) as pt,
            tc.tile_pool(name="ps", bufs=8, space="PSUM") as psum,
        ):
            coef_t = pw.tile([128, 10], F32, tag="coef")
            nc.sync.dma_start(out=coef_t[:, :], in_=coef_ext[:, :])
            w1_t = pw.tile([128, 18 * 256], FP8, tag="w1")
            nc.sync.dma_start(out=w1_t[:, :], in_=w1_ext[:, :])
            w2_t = pw.tile([128, 9 * 256], FP8, tag="w2")
            nc.sync.dma_start(out=w2_t[:, :], in_=w2_ext[:, :])

            st = [dict() for _ in range(BL)]

            def sign_half(i, sv, src, bias_col, h):
                nc.vector.memset(sv[:, h, 0:PW + 1], 0)
                nc.vector.memset(sv[:, h, 65 * PW:HS], 0)
                lc = sv[:, h, 2 * PW - 1:2 * PW - 1 + 64 * PW].rearrange(
                    "p (r c) -> p r c", c=PW)
                nc.vector.memset(lc[:, :, 0:2], 0)
                nc.vector.tensor_scalar(
                    sv[:, h, PW:PW + 64 * PW].rearrange(
                        "p (r c) -> p r c", c=PW)[:, :, 1:1 + W],
                    src[:, :].rearrange("p (y x) -> p y x", y=H),
                    coef_t[:, bias_col + h:bias_col + h + 1], 0.5,
                    op0=Alu.is_ge, op1=Alu.subtract)

            def sign_stage(i, src_tiles, bias_col, tag):
                # s = (src >= -bias) - 0.5  -> {-0.5, +0.5} fp8
                s = psn.tile([128, 2 * HS], FP8, tag="s", name=f"s_{tag}_{i}")
                sv = s[:, :].rearrange("p (h q) -> p h q", h=2, q=HS)
                for h in range(2):
                    sign_half(i, sv, src_tiles[h], bias_col, h)
                st[i][tag] = sv

            def stage_A(i):
                xs = []
                for h in range(2):
                    xb = px.tile([128, HW_], FP16, tag="x", name=f"x_{i}_{h}")
                    nc.sync.dma_start(
                        out=xb[:, :],
                        in_=x_ext[i, h * 128:(h + 1) * 128, :, :].rearrange(
                            "c y x -> c (y x)"),
                    )
                    xs.append(xb)
                st[i]["x"] = xs
                sign_stage(i, xs, 2, "s1")

            def conv(i, sv, w_t, ngrp, drain, post_group=None):
                # tiles grouped 4/4/2 so each stationary weight feeds 4 MMs
                for g in range(ngrp):
                    for tb in (TILES[0:4], TILES[4:8], TILES[8:10]):
                        pts = []
                        for q, (y0, rows) in enumerate(tb):
                            pts.append(psum.tile([128, 512], F32, tag="ps",
                                                 name=f"ps_{i}_{g}_{y0}"))
                        for t in range(9):
                            if probe in ('nomm', 'justdma'):
                                break
                            ky, kx = t // 3, t % 3
                            wap = w_t[:, (g * 9 + t) * 256:(g * 9 + t + 1) * 256
                                      ].rearrange("p (h m) -> p h m", h=2)
                            for q, (y0, rows) in enumerate(tb):
                                n = rows * PW
                                off = PW * (y0 + ky)
                                nc.tensor.matmul(
                                    pts[q][:, 2 - kx:2 - kx + n], wap,
                                    sv[:, :, off:off + n],
                                    start=(t == 0), stop=(t == 8),
                                    perf_mode=DR,
                                )
                        for q, (y0, rows) in enumerate(tb):
                            drain(g, y0, rows, pts[q])
                    if post_group is not None:
                        post_group(g)

            def stage_B(i):  # conv1 + xres (in place into xb)
                xs = st[i]["x"]

                def drain(g, y0, rows, ps):
                    if probe in ('nodrain', 'nomm', 'justdma'):
                        return
                    n = rows * W
                    t1 = pt.tile([128, 448], FP16, tag="t1")
                    src = ps[:, 1:1 + rows * PW].rearrange(
                        "p (r c) -> p r c", c=PW)[:, :, 1:1 + W]
                    nc.scalar.mul(
                        t1[:, :n].rearrange("p (r c) -> p r c", c=W),
                        src, coef_t[:, g:g + 1])
                    xg = xs[g][:, y0 * W:y0 * W + n]
                    nc.vector.tensor_add(xg, xg, t1[:, :n])

                # s2 tile: each half is signed as soon as its group's
                # drains complete, overlapping the other group's matmuls
                s2 = psn.tile([128, 2 * HS], FP8, tag="s", name=f"s_s2_{i}")
                st[i]["s2"] = s2[:, :].rearrange("p (h q) -> p h q", h=2, q=HS)

                def post_group(g):
                    sign_half(i, st[i]["s2"], xs[g], 4, g)

                conv(i, st[i]["s1"], w1_t, 2, drain, post_group)

            def stage_C(i):
                pass

            def stage_D(i):  # conv2 + epilogue + out DMA
                x0 = st[i]["x"][0]

                def drain(g, y0, rows, ps):
                    if probe in ('nodrain', 'noepi', 'nomm', 'justdma'):
                        return
                    n = rows * W
                    t2 = pt.tile([128, 448], FP16, tag="t1")
                    src = ps[:, 1:1 + rows * PW].rearrange(
                        "p (r c) -> p r c", c=PW)[:, :, 1:1 + W]
                    nc.scalar.activation(
                        t2[:, :n].rearrange("p (r c) -> p r c", c=W),
                        src, Ident, bias=coef_t[:, 7:8], scale=coef_t[:, 6:7])
                    xb = x0[:, y0 * W:y0 * W + n]
                    nc.vector.tensor_add(xb, xb, t2[:, :n])   # u = t2 + xres
                    nc.scalar.activation(xb, xb,
                                         mybir.ActivationFunctionType.Prelu,
                                         alpha=coef_t[:, 8:9])  # u = prelu(u)

                conv(i, st[i]["s2"], w2_t, 1, drain)

                uv = x0[:, :].rearrange("p (h2 r1 w2 r2) -> p r1 r2 h2 w2",
                                        h2=32, r1=2, w2=32, r2=2)
                od = out_ext[i, :, :, :].rearrange("(c j) y x -> c j y x", j=4)
                for j in range(4):
                    r1, r2 = j >> 1, j & 1
                    y = pt.tile([128, 1024], F32, tag="y")
                    nc.scalar.activation(
                        y[:, :].rearrange("p (a b) -> p a b", a=32, b=32),
                        uv[:, r1, r2, :, :], Ident,
                        bias=coef_t[:, 9:10], scale=1.0)
                    nc.sync.dma_start(
                        out=od[:, j, :, :],
                        in_=y[:, :].rearrange("p (a b) -> p a b", a=32, b=32))

            # software-pipelined emission: keep the PE busy across images
            for _ in range(reps):
                stage_A(0)
                stage_A(1)
                stage_B(0)
                stage_C(0)
                stage_B(1)
                stage_C(1)
                stage_D(0)
                stage_D(1)

    nc.compile()
    return nc


def _prep_weights(inputs):
    w1 = np.asarray(inputs["conv1_w"], np.float32)          # [256,256,3,3]
    w2 = np.asarray(inputs["conv2_w"], np.float32)          # [128,256,3,3]
    # activations are +-0.5 (not +-1), so conv scales carry an extra 2x
    sc1 = (2.0 * np.abs(w1).mean(axis=(1, 2, 3))
           * float(np.asarray(inputs["kw1"]))
           * float(np.asarray(inputs["ka1"]))).astype(np.float32)   # [256]
    sc2 = (2.0 * np.abs(w2).mean(axis=(1, 2, 3))
           * float(np.asarray(inputs["kw2"]))
           * float(np.asarray(inputs["ka2"]))).astype(np.float32)   # [128]

    # w1b[i, g, t, h, o] = sign(w1)[g*128+o, h*128+i, t//3, t%3]
    sgn1 = np.sign(w1).reshape(2, 128, 2, 128, 9)           # [g,o,h,i,t]
    w1b = np.ascontiguousarray(sgn1.transpose(3, 0, 4, 2, 1)
                               ).reshape(128, 18 * 256).astype(
                                   ml_dtypes.float8_e4m3fn)
    sgn2 = np.sign(w2).reshape(128, 2, 128, 9)              # [o,h,i,t]
    w2b = np.ascontiguousarray(sgn2.transpose(2, 3, 1, 0)
                               ).reshape(128, 9 * 256).astype(
                                   ml_dtypes.float8_e4m3fn)

    coef = np.zeros((128, 10), np.float32)
    coef[:, 0] = sc1[:128]
    coef[:, 1] = sc1[128:]
    b1 = np.asarray(inputs["bias1_"], np.float32).reshape(C)
    coef[:, 2] = -b1[:128]        # is_ge threshold = -bias
    coef[:, 3] = -b1[128:]
    b2 = np.asarray(inputs["bias2_"], np.float32).reshape(C)
    coef[:, 4] = -b2[:128]
    coef[:, 5] = -b2[128:]
    coef[:, 6] = sc2
    coef[:, 7] = np.asarray(inputs["bias3"], np.float32).reshape(C // 2)
    coef[:, 8] = np.asarray(inputs["prelu2_w"], np.float32)
    coef[:, 9] = np.asarray(inputs["bias4"], np.float32).reshape(C // 2)
    return w1b, w2b, coef


def kernel(**inputs):
    return kernel_with_results(**inputs)[0]


def kernel_with_results(trace=False, **inputs):
    x = np.ascontiguousarray(np.asarray(inputs["x"], np.float32).astype(np.float16))
    w1b, w2b, coef = _prep_weights(inputs)

    if "nc" not in _CACHE:
        _CACHE["nc"] = build_nc()
    nc = _CACHE["nc"]

    in_maps = [
        {"x": x[i * BL:(i + 1) * BL], "w1": w1b, "w2": w2b, "coef": coef}
        for i in range(NCORES)
    ]
    res = run_bass_kernel_spmd(nc, in_maps, core_ids=list(range(NCORES)),
                               trace=trace)
    out = np.concatenate([res.results[i]["out"] for i in range(NCORES)], axis=0)
    return out, res

